# revision 38
# baseline (speedup 1.0000x reference)
"""CVRP decoder kernel for Trainium2 (8 NeuronCores, batch-data-parallel).

Computes, per batch b (B=64, P=64, N=1000, H=128):
    q_graph   = mean_n(emb) @ Wq_graph
    q_first   = encoded_q1 @ Wq_first
    q_last    = emb[last_node] @ Wq_last
    q_visited = (vis01 @ emb / N) @ W_visited          (vis01 = isneginf(mask))
    final_q   = sum of the above + load*W_load + b_load
    score     = final_q @ emb^T / sqrt(H) - dists[last_node] / sqrt(2)
    probs     = softmax(10*tanh(score) + (-BIG if visited))

Sharding: batch dim across the 8 cores (pure data parallel), 8 batches per
core processed as 4 pairs of 2 batches stacked on the 128 SBUF partitions.

v3: bf16 matmul path; host-pretiled/pretransposed layouts for dense DMA
packets and single-issue bulk loads; mask folded into the gathered distance
rows off the critical chain (tanh saturation makes exp(10*tanh) ~ 0 for
visited nodes, error ~1e-6 of scale); the mean rides the visited-sum matmul
via a ones-column; the whole working set is SBUF-resident and stages are
emitted in a pair/stage wavefront so the in-order engine queues pipeline
across pairs.
"""

import json
import math
import numpy as np
import ml_dtypes
from contextlib import ExitStack

import concourse.bass as bass
import concourse.mybir as mybir
import concourse.tile as tile
from concourse.bass_utils import run_bass_kernel_spmd
from concourse.masks import make_identity

BF16 = ml_dtypes.bfloat16


def _split_excess_waits(bir_bytes: bytes, max_waits: int = 1) -> bytes:
    """Walrus in this image rejects instructions carrying too many sem waits
    ("Too many sync wait commands", e.g. on Tile's kernel-tail Drain).
    Hoist excess waits onto preceding same-engine EventSemaphore carriers
    (pure sync ops) — sems are monotonic, so a chain of instructions whose
    waits partition the original list is equivalent."""
    d = json.loads(bir_bytes)
    n = [0]
    for fn in d.get("functions", []):
        for blk in fn.get("blocks", []):
            out = []
            for ins in blk.get("instructions", []):
                si = ins.get("sync_info") or {}
                waits = si.get("on_wait") or []
                if len(waits) > max_waits:
                    extra, keep = waits[:-max_waits], waits[-max_waits:]
                    ins["sync_info"]["on_wait"] = keep
                    for i in range(0, len(extra), max_waits):
                        n[0] += 1
                        carrier = {
                            "name": f"I-waitsplit-{n[0]}",
                            "opcode": "EventSemaphore",
                            "engine": ins["engine"],
                            "ins": [],
                            "outs": [],
                            "sync_info": {
                                "on_update": [],
                                "on_wait": extra[i:i + max_waits],
                            },
                        }
                        if "debug" in ins:
                            carrier["debug"] = ins["debug"]
                        out.append(carrier)
                out.append(ins)
            blk["instructions"] = out
    return json.dumps(d).encode()


def _install_walrus_shim():
    import concourse.bass2jax as b2j
    import concourse.bass_utils as bu
    if getattr(bu, "_waitsplit_installed", False):
        return
    real = bu.compile_bir_kernel

    def patched(bir_json, tmpdir, neff_name="file.neff", **kw):
        if isinstance(bir_json, (bytes, bytearray, str)):
            if isinstance(bir_json, str):
                bir_json = bir_json.encode()
            bir_json = _split_excess_waits(bir_json)
        return real(bir_json, tmpdir, neff_name=neff_name, **kw)

    bu.compile_bir_kernel = patched
    b2j.compile_bir_kernel = patched
    bu._waitsplit_installed = True


_install_walrus_shim()

F32 = mybir.dt.float32
F8 = mybir.dt.float8e4
F16 = mybir.dt.float16
BF = mybir.dt.bfloat16
I32 = mybir.dt.int32
OP = mybir.AluOpType
AF = mybir.ActivationFunctionType

B, P, N, H = 64, 64, 1000, 128
NCORES = 8
NB = B // NCORES          # 8 batches per core
NPAIR = NB // 2           # 4 pairs
NC = 8                    # n-chunks of 128 rows (last padded 104->128)
NPAD = NC * 128           # 1024
PAIR_ORDER = [0, 2, 1, 3]  # DMA arrival order across the two hwdge rings

MASK_QV = -128.0          # maskT encoding for the visited-sum matmul
QV_SCALE = -1.0 / (128.0 * N)
MASK_PRE = 30.0           # added to dist rows: tanh(score - 21.2) -> -1
MEAN_SCALE = 1.0 / N
FQ_SCALE = math.sqrt(2.0) / math.sqrt(H)   # = 0.125 exactly
TANH_SCALE = 1.0 / math.sqrt(2.0)
TANH_CLIP = 10.0


def build_nc():
    nc = bass.Bass()

    dists = nc.dram_tensor("dists", [NB * N, N], F16, kind="ExternalInput")
    embt = nc.dram_tensor("embt", [128, NB * NC * H], BF,
                          kind="ExternalInput")
    maskT = nc.dram_tensor("maskT", [128, NB * NC * 66], F8,
                           kind="ExternalInput")
    maskn = nc.dram_tensor("maskn", [128, NPAIR * N], F16,
                           kind="ExternalInput")
    c_big = nc.dram_tensor("c_big", [128, 1536], BF, kind="ExternalInput")
    c_row = nc.dram_tensor("c_row", [1, 768], BF, kind="ExternalInput")
    idxt = nc.dram_tensor("idxt", [128, NPAIR], I32, kind="ExternalInput")
    probs = nc.dram_tensor("probs", [NB * P, N], BF, kind="ExternalOutput")

    with tile.TileContext(nc) as tc:
        with ExitStack() as ctx:
            const = ctx.enter_context(tc.tile_pool(name="const", bufs=1))
            sb = ctx.enter_context(tc.tile_pool(name="sb", bufs=1))
            ps_T = ctx.enter_context(
                tc.tile_pool(name="ps_T", bufs=3, space="PSUM"))
            ps_big = ctx.enter_context(
                tc.tile_pool(name="ps_big", bufs=2, space="PSUM"))
            ps_qv = ctx.enter_context(
                tc.tile_pool(name="ps_qv", bufs=2, space="PSUM"))
            ps_fq = ctx.enter_context(
                tc.tile_pool(name="ps_fq", bufs=1, space="PSUM"))

            # ---- constants / bulk params ----
            ident = const.tile([128, 128], BF, tag="ident")
            make_identity(nc, ident[:])
            ones_row = const.tile([1, 128], BF, tag="ones_row")
            nc.gpsimd.memset(ones_row[:], 1.0)
            cbig = const.tile([128, 1536], BF, tag="cbig")
            nc.scalar.dma_start(cbig[:], c_big[:])
            crow = const.tile([1, 768], BF, tag="crow")
            nc.scalar.dma_start(crow[:], c_row[:])
            idxs = const.tile([128, NPAIR], I32, tag="idxs")
            nc.gpsimd.dma_start(idxs[:], idxt[:])
            wall = cbig[:, 0:512].rearrange("k (i h) -> k i h", i=4)
            eqall = cbig[:, 512:1024].rearrange("k (p h) -> k p h", p=NPAIR)
            leT = cbig[:, 1024:1536].rearrange("k (p h) -> k p h", p=NPAIR)
            wlb = crow[:, 0:256].rearrange("k (i h) -> k i h", i=2)
            ldall = crow[:, 256:768].rearrange("k (p h) -> k p h", p=NPAIR)

            # ---- SBUF-resident working set ----
            et_all = sb.tile([128, NB, NC, H], BF, tag="et_all",
                             name="et_all")
            mT_all = sb.tile([128, NB, NC, 66], F8, tag="mT_all",
                             name="mT_all")
            dmb_all = sb.tile([128, NPAIR, N], F16, tag="dmb_all",
                              name="dmb_all")
            et = [et_all[:, 2 * pr:2 * pr + 2] for pr in range(NPAIR)]
            mT = [mT_all[:, 2 * pr:2 * pr + 2] for pr in range(NPAIR)]
            dmb = [dmb_all[:, pr] for pr in range(NPAIR)]

            eT = [[sb.tile([128, NPAD], BF, tag=f"eT{pr}_{j}", name=f"eT{pr}_{j}")
                   for j in range(2)] for pr in range(NPAIR)]
            fqr = [sb.tile([128, 2, 128], BF, tag=f"fqr{pr}", name=f"fqr{pr}")
                   for pr in range(NPAIR)]
            fqT = [sb.tile([128, 128], BF, tag=f"fqT{pr}", name=f"fqT{pr}")
                   for pr in range(NPAIR)]
            u = [sb.tile([128, N], F16, tag=f"u{pr}", name=f"u{pr}") for pr in range(NPAIR)]
            t = [sb.tile([128, N], F32, tag=f"t{pr}", name=f"t{pr}") for pr in range(NPAIR)]
            e = [sb.tile([128, N], F16, tag=f"e{pr}", name=f"e{pr}") for pr in range(NPAIR)]
            ssum = [sb.tile([128, 1], F32, tag=f"ssum{pr}", name=f"ssum{pr}")
                    for pr in range(NPAIR)]
            rec = [sb.tile([128, 1], F32, tag=f"rec{pr}", name=f"rec{pr}")
                   for pr in range(NPAIR)]
            pout = [sb.tile([128, N], BF, tag=f"pout{pr}", name=f"pout{pr}")
                    for pr in range(NPAIR)]

            # ---- pure load prologue: every DMA issued before any compute,
            # so no load issue ever queues behind compute on its engine
            # pairs 0,1 grouped on the sync ring; pairs 2,3 follow the
            # consts on the scalar ring -> arrival order [0, 2, 1, 3]
            for pr in range(NPAIR):
                b0 = 2 * pr
                eng = nc.sync if pr < 2 else nc.scalar
                eng.dma_start(
                    dmb_all[:, pr], maskn[:, pr * N:(pr + 1) * N])
                eng.dma_start(
                    et[pr],
                    embt[:, b0 * NC * H:(b0 + 2) * NC * H]
                    .rearrange("k (j c h) -> k j c h", j=2, c=NC))
                eng.dma_start(
                    mT[pr],
                    maskT[:, b0 * NC * 66:(b0 + 2) * NC * 66]
                    .rearrange("k (j c h) -> k j c h", j=2, c=NC))
            for pr in PAIR_ORDER:
                nc.gpsimd.indirect_dma_start(
                    out=dmb_all[:, pr], out_offset=None, in_=dists[:],
                    in_offset=bass.IndirectOffsetOnAxis(
                        ap=idxs[:, pr:pr + 1], axis=0),
                    compute_op=OP.add)

            def stage_transpose(pr):
                for j in range(2):
                    psA = ps_T.tile([128, 512], BF, tag="bigT")
                    for c in range(4):
                        nc.tensor.transpose(
                            out=psA[:, 128 * c:128 * (c + 1)],
                            in_=et[pr][:, j, c, :], identity=ident[:])
                    psB = ps_T.tile([128, 512], BF, tag="bigT")
                    for c in range(4, NC):
                        nc.tensor.transpose(
                            out=psB[:, 128 * (c - 4):128 * (c - 3)],
                            in_=et[pr][:, j, c, :], identity=ident[:])
                    if j == 0:
                        nc.vector.tensor_copy(out=eT[pr][j][:, 0:512],
                                              in_=psA[:])
                        nc.vector.tensor_copy(out=eT[pr][j][:, 512:1024],
                                              in_=psB[:])
                    else:
                        nc.vector.tensor_copy(out=eT[pr][j][:, 0:512],
                                              in_=psA[:])
                        nc.vector.tensor_copy(out=eT[pr][j][:, 512:1024],
                                              in_=psB[:])

            def stage_qv(pr):
                qv = ps_qv.tile([128, 2, 65], F32, tag="qv")
                for j in range(2):
                    for c in range(NC):
                        nc.tensor.matmul(
                            qv[:, j, :],
                            lhsT=et[pr][:, j, c, :],
                            rhs=mT[pr][:, j, c, 0:65],
                            start=(c == 0), stop=(c == NC - 1))
                nc.vector.tensor_scalar(
                    out=fqr[pr][:, 1, :].rearrange("k (j p) -> k j p", j=2),
                    in0=qv[:, :, 0:64], scalar1=QV_SCALE,
                    scalar2=None, op0=OP.mult)
                nc.vector.tensor_scalar(
                    out=fqr[pr][:, 0, :].rearrange("k (j p) -> k j p", j=2),
                    in0=qv[:, :, 64:65].to_broadcast([128, 2, 64]),
                    scalar1=MEAN_SCALE, scalar2=None, op0=OP.mult)

            def stage_fq(pr):
                pfq = ps_fq.tile([128, 128], F32, tag="pfq")
                nc.tensor.matmul(pfq[:], lhsT=wall[:, 0, :],
                                 rhs=eqall[:, pr, :], start=True, stop=False)
                nc.tensor.matmul(pfq[:], lhsT=wall[:, 1, :],
                                 rhs=leT[:, pr, :], start=False, stop=False)
                nc.tensor.matmul(pfq[:], lhsT=wall[:, 2, :],
                                 rhs=fqr[pr][:, 0, :], start=False, stop=False)
                nc.tensor.matmul(pfq[:], lhsT=wall[:, 3, :],
                                 rhs=fqr[pr][:, 1, :], start=False, stop=False)
                nc.tensor.matmul(pfq[:], lhsT=wlb[:, 0, :],
                                 rhs=ldall[:, pr, :], start=False, stop=False)
                nc.tensor.matmul(pfq[:], lhsT=wlb[:, 1, :], rhs=ones_row[:],
                                 start=False, stop=True)
                nc.vector.tensor_scalar(
                    out=fqT[pr][:], in0=pfq[:], scalar1=FQ_SCALE,
                    scalar2=None, op0=OP.mult)

            def stage_score(pr):
                for (n0, n1) in ((0, 512), (512, N)):
                    psc = ps_big.tile([128, 512], F32, tag="big")
                    for j in range(2):
                        nc.tensor.matmul(
                            psc[64 * j:64 * j + 64, 0:n1 - n0],
                            lhsT=fqT[pr][:, 64 * j:64 * j + 64],
                            rhs=eT[pr][j][:, n0:n1],
                            start=True, stop=True)
                    nc.vector.scalar_tensor_tensor(
                        out=u[pr][:, n0:n1], in0=psc[:, 0:n1 - n0],
                        scalar=0.0, in1=dmb[pr][:, n0:n1],
                        op0=OP.bypass, op1=OP.subtract)

            def stage_exp(pr):
                nc.scalar.activation(t[pr][:], u[pr][:], AF.Tanh,
                                     scale=TANH_SCALE)
                nc.scalar.activation(e[pr][:], t[pr][:], AF.Exp,
                                     scale=TANH_CLIP, accum_out=ssum[pr][:])
                nc.vector.reciprocal(out=rec[pr][:], in_=ssum[pr][:])

            def stage_out(pr):
                nc.vector.tensor_scalar(
                    out=pout[pr][:], in0=e[pr][:], scalar1=rec[pr][:, 0:1],
                    scalar2=None, op0=OP.mult)
                nc.sync.dma_start(probs[128 * pr:128 * pr + 128, :],
                                  pout[pr][:])

            # phase A: data-driven stages, wavefronted by DMA arrival order
            stagesA = [stage_transpose, stage_qv, stage_fq, stage_score]
            for d in range(NPAIR - 1 + len(stagesA)):
                for k in range(NPAIR):
                    s = d - k
                    if 0 <= s < len(stagesA):
                        stagesA[s](PAIR_ORDER[k])
            # phase B: the ACT-serial softmax chain, pair-major at the end so
            # no late pair's eviction queues behind an early pair's exp
            for pr in PAIR_ORDER:
                stage_exp(pr)
                stage_out(pr)

    return nc


_CACHE = {}


def _get_nc():
    if "nc" not in _CACHE:
        _CACHE["nc"] = build_nc()
    return _CACHE["nc"]


def _prep_inputs(inputs):
    """Host-side staging: dtype casts + DMA-friendly layouts (per full batch,
    then sliced per core)."""
    emb = np.ascontiguousarray(inputs["embeddings"], dtype=np.float32)
    emb_bf = emb.astype(BF16)                              # [B, N, H]
    # tiled: [B, 128, NC, H], row 128c+k -> [k, c], zero-padded past N
    emb_pad = np.zeros((B, NPAD, H), dtype=BF16)
    emb_pad[:, :N, :] = emb_bf
    embt = np.ascontiguousarray(
        emb_pad.reshape(B, NC, 128, H).transpose(0, 2, 1, 3))  # [B,128,NC,H]

    mask = np.ascontiguousarray(inputs["group_ninf_mask"], dtype=np.float32)
    vis = (mask < -1e30)
    F8NP = ml_dtypes.float8_e4m3
    enc = np.where(vis, np.float32(MASK_QV), np.float32(0)).astype(F8NP)
    # transposed+tiled with ones column: [B, 128, NC, 66]
    enc_pad = np.zeros((B, P, NPAD), dtype=F8NP)
    enc_pad[:, :, :N] = enc
    mt = enc_pad.reshape(B, P, NC, 128).transpose(0, 3, 2, 1)  # [B,128,NC,P]
    ones_col = np.zeros((B, 128, NC, 1), dtype=F8NP)
    rowidx = np.arange(128)[:, None] + np.arange(NC)[None, :] * 128  # [128,NC]
    ones_col[:, :, :, 0] = (rowidx < N).astype(F8NP)[None, :, :]
    zero_col = np.zeros((B, 128, NC, 1), dtype=F8NP)
    maskTh = np.ascontiguousarray(
        np.concatenate([mt, ones_col, zero_col], axis=3))   # [B,128,NC,66]

    # pre-tanh mask bias, rides on the dist rows: {0, +MASK_PRE} fp16
    mknpre = np.where(vis, np.float16(MASK_PRE),
                      np.float16(0))                        # [B, P, N] f16

    q1 = np.ascontiguousarray(inputs["encoded_q1"], dtype=np.float32)
    q1_bf = q1.astype(BF16)                                 # [B, P, H]
    # per pair of batches: [h, 2*64]
    eq1T = np.ascontiguousarray(
        q1_bf.reshape(B // 2, 2, P, H).transpose(0, 3, 1, 2)
        .reshape(B // 2, H, 2 * P))                         # [B/2,128,128]

    last = np.ascontiguousarray(inputs["last_node"]).astype(np.int64)
    # last-node embedding rows, host-gathered from the bf16 copy, transposed
    le = np.take_along_axis(emb_bf, last[:, :, None], axis=1)  # [B, P, H]
    leT = np.ascontiguousarray(
        le.reshape(B // 2, 2, P, H).transpose(0, 3, 1, 2)
        .reshape(B // 2, H, 2 * P))                         # [B/2,128,128]
    dists = np.ascontiguousarray(inputs["dists"], dtype=np.float32)
    dists_h = dists.astype(np.float16)                      # [B, N, N]

    load = np.ascontiguousarray(inputs["load"], dtype=np.float32).astype(BF16)

    w_bf = {k: np.ascontiguousarray(inputs[k], dtype=np.float32).astype(BF16)
            for k in ("Wq_graph", "Wq_first", "Wq_last", "W_visited",
                      "W_load", "b_load")}
    # [h, (wf, wl, wg, wv)] stacking for the single W_all tile
    w_stack = np.ascontiguousarray(np.stack(
        [w_bf["Wq_first"], w_bf["Wq_last"], w_bf["Wq_graph"],
         w_bf["W_visited"]], axis=1))                       # [H, 4, H]
    w_lb = np.ascontiguousarray(np.stack(
        [w_bf["W_load"], w_bf["b_load"]], axis=0))[None]    # [1, 2, H]

    in_maps = []
    for c in range(NCORES):
        s = slice(c * NB, (c + 1) * NB)
        lastc = last[s]                                     # [NB, P]
        # flat gather index within the core slab: n + N*local_b
        idx = (lastc + (np.arange(NB) * N)[:, None]).astype(np.int32)
        idxt = np.ascontiguousarray(
            idx.reshape(NPAIR, 128).T)                      # [128, NPAIR]
        eqc = np.ascontiguousarray(
            eq1T[c * NPAIR:(c + 1) * NPAIR].transpose(1, 0, 2)).reshape(
            128, NPAIR * 128)
        lec = np.ascontiguousarray(
            leT[c * NPAIR:(c + 1) * NPAIR].transpose(1, 0, 2)).reshape(
            128, NPAIR * 128)
        cbig = np.ascontiguousarray(np.concatenate(
            [w_stack.reshape(128, 4 * H), eqc, lec], axis=1))
        crow = np.ascontiguousarray(np.concatenate(
            [w_lb.reshape(1, 2 * H), load[s].reshape(1, NPAIR * 128)],
            axis=1))
        in_maps.append(dict(
            dists=dists_h[s].reshape(NB * N, N),
            embt=np.ascontiguousarray(
                embt[s].transpose(1, 0, 2, 3)).reshape(128, NB * NC * H),
            maskT=np.ascontiguousarray(
                maskTh[s].transpose(1, 0, 2, 3)).reshape(128, NB * NC * 66),
            maskn=np.ascontiguousarray(
                mknpre[s].reshape(NPAIR, 128, N).transpose(1, 0, 2))
                .reshape(128, NPAIR * N),
            c_big=cbig,
            c_row=crow,
            idxt=idxt,
        ))
    return in_maps


def _run(inputs, trace=False, **kw):
    nc = _get_nc()
    in_maps = _prep_inputs(inputs)
    res = run_bass_kernel_spmd(nc, in_maps, list(range(NCORES)),
                               trace=trace, **kw)
    out = np.concatenate(
        [np.asarray(r["probs"]).astype(np.float32).reshape(NB, P, N)
         for r in res.results], axis=0)
    return out, res


def kernel(**inputs) -> np.ndarray:
    out, _ = _run(inputs)
    return out


# revision 39
# speedup vs baseline: 1.0340x; 1.0340x over previous
"""CVRP decoder kernel for Trainium2 (8 NeuronCores, batch-data-parallel).

Computes, per batch b (B=64, P=64, N=1000, H=128):
    q_graph   = mean_n(emb) @ Wq_graph
    q_first   = encoded_q1 @ Wq_first
    q_last    = emb[last_node] @ Wq_last
    q_visited = (vis01 @ emb / N) @ W_visited          (vis01 = isneginf(mask))
    final_q   = sum of the above + load*W_load + b_load
    score     = final_q @ emb^T / sqrt(H) - dists[last_node] / sqrt(2)
    probs     = softmax(10*tanh(score) + (-BIG if visited))

Sharding: batch dim across the 8 cores (pure data parallel), 8 batches per
core processed as 4 pairs of 2 batches stacked on the 128 SBUF partitions.

v3: bf16 matmul path; host-pretiled/pretransposed layouts for dense DMA
packets and single-issue bulk loads; mask folded into the gathered distance
rows off the critical chain (tanh saturation makes exp(10*tanh) ~ 0 for
visited nodes, error ~1e-6 of scale); the mean rides the visited-sum matmul
via a ones-column; the whole working set is SBUF-resident and stages are
emitted in a pair/stage wavefront so the in-order engine queues pipeline
across pairs.
"""

import json
import math
import numpy as np
import ml_dtypes
from contextlib import ExitStack

import concourse.bass as bass
import concourse.mybir as mybir
import concourse.tile as tile
from concourse.bass_utils import run_bass_kernel_spmd
from concourse.masks import make_identity

BF16 = ml_dtypes.bfloat16


def _split_excess_waits(bir_bytes: bytes, max_waits: int = 1) -> bytes:
    """Walrus in this image rejects instructions carrying too many sem waits
    ("Too many sync wait commands", e.g. on Tile's kernel-tail Drain).
    Hoist excess waits onto preceding same-engine EventSemaphore carriers
    (pure sync ops) — sems are monotonic, so a chain of instructions whose
    waits partition the original list is equivalent."""
    d = json.loads(bir_bytes)
    n = [0]
    for fn in d.get("functions", []):
        for blk in fn.get("blocks", []):
            out = []
            for ins in blk.get("instructions", []):
                si = ins.get("sync_info") or {}
                waits = si.get("on_wait") or []
                if len(waits) > max_waits:
                    extra, keep = waits[:-max_waits], waits[-max_waits:]
                    ins["sync_info"]["on_wait"] = keep
                    for i in range(0, len(extra), max_waits):
                        n[0] += 1
                        carrier = {
                            "name": f"I-waitsplit-{n[0]}",
                            "opcode": "EventSemaphore",
                            "engine": ins["engine"],
                            "ins": [],
                            "outs": [],
                            "sync_info": {
                                "on_update": [],
                                "on_wait": extra[i:i + max_waits],
                            },
                        }
                        if "debug" in ins:
                            carrier["debug"] = ins["debug"]
                        out.append(carrier)
                out.append(ins)
            blk["instructions"] = out
    return json.dumps(d).encode()


def _install_walrus_shim():
    import concourse.bass2jax as b2j
    import concourse.bass_utils as bu
    if getattr(bu, "_waitsplit_installed", False):
        return
    real = bu.compile_bir_kernel

    def patched(bir_json, tmpdir, neff_name="file.neff", **kw):
        if isinstance(bir_json, (bytes, bytearray, str)):
            if isinstance(bir_json, str):
                bir_json = bir_json.encode()
            bir_json = _split_excess_waits(bir_json)
        return real(bir_json, tmpdir, neff_name=neff_name, **kw)

    bu.compile_bir_kernel = patched
    b2j.compile_bir_kernel = patched
    bu._waitsplit_installed = True


_install_walrus_shim()

F32 = mybir.dt.float32
F8 = mybir.dt.float8e4
F16 = mybir.dt.float16
BF = mybir.dt.bfloat16
I32 = mybir.dt.int32
OP = mybir.AluOpType
AF = mybir.ActivationFunctionType

B, P, N, H = 64, 64, 1000, 128
NCORES = 8
NB = B // NCORES          # 8 batches per core
NPAIR = NB // 2           # 4 pairs
NC = 8                    # n-chunks of 128 rows (last padded 104->128)
NPAD = NC * 128           # 1024
PAIR_ORDER = [0, 2, 1, 3]  # DMA arrival order across the two hwdge rings

MASK_QV = -128.0          # maskT encoding for the visited-sum matmul
QV_SCALE = -1.0 / (128.0 * N)
MASK_PRE = 30.0           # added to dist rows: tanh(score - 21.2) -> -1
MEAN_SCALE = 1.0 / N
FQ_SCALE = math.sqrt(2.0) / math.sqrt(H)   # = 0.125 exactly
TANH_SCALE = 1.0 / math.sqrt(2.0)
TANH_CLIP = 10.0


def build_nc():
    nc = bass.Bass()

    dists = nc.dram_tensor("dists", [NB * N, N], F16, kind="ExternalInput")
    embt = nc.dram_tensor("embt", [128, NB * NC * H], BF,
                          kind="ExternalInput")
    maskT = nc.dram_tensor("maskT", [128, NB * NC * 66], F8,
                           kind="ExternalInput")
    maskn = nc.dram_tensor("maskn", [128, NPAIR * N], F16,
                           kind="ExternalInput")
    c_big = nc.dram_tensor("c_big", [128, 1536], BF, kind="ExternalInput")
    c_row = nc.dram_tensor("c_row", [1, 768], BF, kind="ExternalInput")
    idxt = nc.dram_tensor("idxt", [128, NPAIR], I32, kind="ExternalInput")
    probs = nc.dram_tensor("probs", [NB * P, N], BF, kind="ExternalOutput")

    with tile.TileContext(nc) as tc:
        with ExitStack() as ctx:
            const = ctx.enter_context(tc.tile_pool(name="const", bufs=1))
            sb = ctx.enter_context(tc.tile_pool(name="sb", bufs=1))
            ps_T = ctx.enter_context(
                tc.tile_pool(name="ps_T", bufs=3, space="PSUM"))
            ps_big = ctx.enter_context(
                tc.tile_pool(name="ps_big", bufs=2, space="PSUM"))
            ps_qv = ctx.enter_context(
                tc.tile_pool(name="ps_qv", bufs=2, space="PSUM"))
            ps_fq = ctx.enter_context(
                tc.tile_pool(name="ps_fq", bufs=1, space="PSUM"))

            # ---- constants / bulk params ----
            ident = const.tile([128, 128], BF, tag="ident")
            make_identity(nc, ident[:])
            ones_row = const.tile([1, 128], BF, tag="ones_row")
            nc.gpsimd.memset(ones_row[:], 1.0)
            cbig = const.tile([128, 1536], BF, tag="cbig")
            nc.scalar.dma_start(cbig[:], c_big[:])
            crow = const.tile([1, 768], BF, tag="crow")
            nc.scalar.dma_start(crow[:], c_row[:])
            idxs = const.tile([128, NPAIR], I32, tag="idxs")
            nc.gpsimd.dma_start(idxs[:], idxt[:])
            wall = cbig[:, 0:512].rearrange("k (i h) -> k i h", i=4)
            eqall = cbig[:, 512:1024].rearrange("k (p h) -> k p h", p=NPAIR)
            leT = cbig[:, 1024:1536].rearrange("k (p h) -> k p h", p=NPAIR)
            wlb = crow[:, 0:256].rearrange("k (i h) -> k i h", i=2)
            ldall = crow[:, 256:768].rearrange("k (p h) -> k p h", p=NPAIR)

            # ---- SBUF-resident working set ----
            et_all = sb.tile([128, NB, NC, H], BF, tag="et_all",
                             name="et_all")
            mT_all = sb.tile([128, NB, NC, 66], F8, tag="mT_all",
                             name="mT_all")
            dmb_all = sb.tile([128, NPAIR, N], F16, tag="dmb_all",
                              name="dmb_all")
            et = [et_all[:, 2 * pr:2 * pr + 2] for pr in range(NPAIR)]
            mT = [mT_all[:, 2 * pr:2 * pr + 2] for pr in range(NPAIR)]
            dmb = [dmb_all[:, pr] for pr in range(NPAIR)]

            eT = [[sb.tile([128, NPAD], BF, tag=f"eT{pr}_{j}", name=f"eT{pr}_{j}")
                   for j in range(2)] for pr in range(NPAIR)]
            fqr = [sb.tile([128, 2, 128], BF, tag=f"fqr{pr}", name=f"fqr{pr}")
                   for pr in range(NPAIR)]
            fqT = [sb.tile([128, 128], BF, tag=f"fqT{pr}", name=f"fqT{pr}")
                   for pr in range(NPAIR)]
            u = [sb.tile([128, N], F16, tag=f"u{pr}", name=f"u{pr}") for pr in range(NPAIR)]
            t = [sb.tile([128, N], F32, tag=f"t{pr}", name=f"t{pr}") for pr in range(NPAIR)]
            e = [sb.tile([128, N], F16, tag=f"e{pr}", name=f"e{pr}") for pr in range(NPAIR)]
            ssum = [sb.tile([128, 1], F32, tag=f"ssum{pr}", name=f"ssum{pr}")
                    for pr in range(NPAIR)]
            rec = [sb.tile([128, 1], F32, tag=f"rec{pr}", name=f"rec{pr}")
                   for pr in range(NPAIR)]
            pout = [sb.tile([128, N], BF, tag=f"pout{pr}", name=f"pout{pr}")
                    for pr in range(NPAIR)]

            # ---- pure load prologue: every DMA issued before any compute,
            # so no load issue ever queues behind compute on its engine
            # pairs 0,1 grouped on the sync ring; pairs 2,3 follow the
            # consts on the scalar ring -> arrival order [0, 2, 1, 3]
            for pr in range(NPAIR):
                b0 = 2 * pr
                eng = nc.sync if pr < 2 else nc.scalar
                eng.dma_start(
                    dmb_all[:, pr], maskn[:, pr * N:(pr + 1) * N])
                eng.dma_start(
                    et[pr],
                    embt[:, b0 * NC * H:(b0 + 2) * NC * H]
                    .rearrange("k (j c h) -> k j c h", j=2, c=NC))
                eng.dma_start(
                    mT[pr],
                    maskT[:, b0 * NC * 66:(b0 + 2) * NC * 66]
                    .rearrange("k (j c h) -> k j c h", j=2, c=NC))
            for pr in PAIR_ORDER:
                nc.gpsimd.indirect_dma_start(
                    out=dmb_all[:, pr], out_offset=None, in_=dists[:],
                    in_offset=bass.IndirectOffsetOnAxis(
                        ap=idxs[:, pr:pr + 1], axis=0),
                    compute_op=OP.add)

            def stage_transpose(pr):
                for j in range(2):
                    psA = ps_T.tile([128, 512], BF, tag="bigT")
                    for c in range(4):
                        nc.tensor.transpose(
                            out=psA[:, 128 * c:128 * (c + 1)],
                            in_=et[pr][:, j, c, :], identity=ident[:])
                    psB = ps_T.tile([128, 512], BF, tag="bigT")
                    for c in range(4, NC):
                        nc.tensor.transpose(
                            out=psB[:, 128 * (c - 4):128 * (c - 3)],
                            in_=et[pr][:, j, c, :], identity=ident[:])
                    if j == 0:
                        nc.vector.tensor_copy(out=eT[pr][j][:, 0:512],
                                              in_=psA[:])
                        nc.vector.tensor_copy(out=eT[pr][j][:, 512:1024],
                                              in_=psB[:])
                    else:
                        nc.vector.tensor_copy(out=eT[pr][j][:, 0:512],
                                              in_=psA[:])
                        nc.vector.tensor_copy(out=eT[pr][j][:, 512:1024],
                                              in_=psB[:])

            def stage_qv(pr):
                qv = ps_qv.tile([128, 2, 65], F32, tag="qv")
                for j in range(2):
                    for c in range(NC):
                        nc.tensor.matmul(
                            qv[:, j, :],
                            lhsT=et[pr][:, j, c, :],
                            rhs=mT[pr][:, j, c, 0:65],
                            start=(c == 0), stop=(c == NC - 1))
                nc.scalar.mul(
                    fqr[pr][:, 1, :].rearrange("k (j p) -> k j p", j=2),
                    qv[:, :, 0:64], QV_SCALE)
                nc.vector.tensor_scalar(
                    out=fqr[pr][:, 0, :].rearrange("k (j p) -> k j p", j=2),
                    in0=qv[:, :, 64:65].to_broadcast([128, 2, 64]),
                    scalar1=MEAN_SCALE, scalar2=None, op0=OP.mult)

            def stage_fq(pr):
                pfq = ps_fq.tile([128, 128], F32, tag="pfq")
                nc.tensor.matmul(pfq[:], lhsT=wall[:, 0, :],
                                 rhs=eqall[:, pr, :], start=True, stop=False)
                nc.tensor.matmul(pfq[:], lhsT=wall[:, 1, :],
                                 rhs=leT[:, pr, :], start=False, stop=False)
                nc.tensor.matmul(pfq[:], lhsT=wall[:, 2, :],
                                 rhs=fqr[pr][:, 0, :], start=False, stop=False)
                nc.tensor.matmul(pfq[:], lhsT=wall[:, 3, :],
                                 rhs=fqr[pr][:, 1, :], start=False, stop=False)
                nc.tensor.matmul(pfq[:], lhsT=wlb[:, 0, :],
                                 rhs=ldall[:, pr, :], start=False, stop=False)
                nc.tensor.matmul(pfq[:], lhsT=wlb[:, 1, :], rhs=ones_row[:],
                                 start=False, stop=True)
                nc.vector.tensor_scalar(
                    out=fqT[pr][:], in0=pfq[:], scalar1=FQ_SCALE,
                    scalar2=None, op0=OP.mult)

            def stage_score(pr):
                for (n0, n1) in ((0, 512), (512, N)):
                    psc = ps_big.tile([128, 512], F32, tag="big")
                    for j in range(2):
                        nc.tensor.matmul(
                            psc[64 * j:64 * j + 64, 0:n1 - n0],
                            lhsT=fqT[pr][:, 64 * j:64 * j + 64],
                            rhs=eT[pr][j][:, n0:n1],
                            start=True, stop=True)
                    nc.vector.scalar_tensor_tensor(
                        out=u[pr][:, n0:n1], in0=psc[:, 0:n1 - n0],
                        scalar=0.0, in1=dmb[pr][:, n0:n1],
                        op0=OP.bypass, op1=OP.subtract)

            def stage_exp(pr):
                nc.scalar.activation(t[pr][:], u[pr][:], AF.Tanh,
                                     scale=TANH_SCALE)
                nc.scalar.activation(e[pr][:], t[pr][:], AF.Exp,
                                     scale=TANH_CLIP, accum_out=ssum[pr][:])
                nc.vector.reciprocal(out=rec[pr][:], in_=ssum[pr][:])

            def stage_out(pr):
                nc.vector.tensor_scalar(
                    out=pout[pr][:], in0=e[pr][:], scalar1=rec[pr][:, 0:1],
                    scalar2=None, op0=OP.mult)
                nc.sync.dma_start(probs[128 * pr:128 * pr + 128, :],
                                  pout[pr][:])

            # phase A: data-driven stages, wavefronted by DMA arrival order
            stagesA = [stage_transpose, stage_qv, stage_fq, stage_score]
            for d in range(NPAIR - 1 + len(stagesA)):
                for k in range(NPAIR):
                    s = d - k
                    if 0 <= s < len(stagesA):
                        stagesA[s](PAIR_ORDER[k])
            # phase B: the ACT-serial softmax chain, pair-major at the end so
            # no late pair's eviction queues behind an early pair's exp
            for pr in PAIR_ORDER:
                stage_exp(pr)
                stage_out(pr)

    return nc


_CACHE = {}


def _get_nc():
    if "nc" not in _CACHE:
        _CACHE["nc"] = build_nc()
    return _CACHE["nc"]


def _prep_inputs(inputs):
    """Host-side staging: dtype casts + DMA-friendly layouts (per full batch,
    then sliced per core)."""
    emb = np.ascontiguousarray(inputs["embeddings"], dtype=np.float32)
    emb_bf = emb.astype(BF16)                              # [B, N, H]
    # tiled: [B, 128, NC, H], row 128c+k -> [k, c], zero-padded past N
    emb_pad = np.zeros((B, NPAD, H), dtype=BF16)
    emb_pad[:, :N, :] = emb_bf
    embt = np.ascontiguousarray(
        emb_pad.reshape(B, NC, 128, H).transpose(0, 2, 1, 3))  # [B,128,NC,H]

    mask = np.ascontiguousarray(inputs["group_ninf_mask"], dtype=np.float32)
    vis = (mask < -1e30)
    F8NP = ml_dtypes.float8_e4m3
    enc = np.where(vis, np.float32(MASK_QV), np.float32(0)).astype(F8NP)
    # transposed+tiled with ones column: [B, 128, NC, 66]
    enc_pad = np.zeros((B, P, NPAD), dtype=F8NP)
    enc_pad[:, :, :N] = enc
    mt = enc_pad.reshape(B, P, NC, 128).transpose(0, 3, 2, 1)  # [B,128,NC,P]
    ones_col = np.zeros((B, 128, NC, 1), dtype=F8NP)
    rowidx = np.arange(128)[:, None] + np.arange(NC)[None, :] * 128  # [128,NC]
    ones_col[:, :, :, 0] = (rowidx < N).astype(F8NP)[None, :, :]
    zero_col = np.zeros((B, 128, NC, 1), dtype=F8NP)
    maskTh = np.ascontiguousarray(
        np.concatenate([mt, ones_col, zero_col], axis=3))   # [B,128,NC,66]

    # pre-tanh mask bias, rides on the dist rows: {0, +MASK_PRE} fp16
    mknpre = np.where(vis, np.float16(MASK_PRE),
                      np.float16(0))                        # [B, P, N] f16

    q1 = np.ascontiguousarray(inputs["encoded_q1"], dtype=np.float32)
    q1_bf = q1.astype(BF16)                                 # [B, P, H]
    # per pair of batches: [h, 2*64]
    eq1T = np.ascontiguousarray(
        q1_bf.reshape(B // 2, 2, P, H).transpose(0, 3, 1, 2)
        .reshape(B // 2, H, 2 * P))                         # [B/2,128,128]

    last = np.ascontiguousarray(inputs["last_node"]).astype(np.int64)
    # last-node embedding rows, host-gathered from the bf16 copy, transposed
    le = np.take_along_axis(emb_bf, last[:, :, None], axis=1)  # [B, P, H]
    leT = np.ascontiguousarray(
        le.reshape(B // 2, 2, P, H).transpose(0, 3, 1, 2)
        .reshape(B // 2, H, 2 * P))                         # [B/2,128,128]
    dists = np.ascontiguousarray(inputs["dists"], dtype=np.float32)
    dists_h = dists.astype(np.float16)                      # [B, N, N]

    load = np.ascontiguousarray(inputs["load"], dtype=np.float32).astype(BF16)

    w_bf = {k: np.ascontiguousarray(inputs[k], dtype=np.float32).astype(BF16)
            for k in ("Wq_graph", "Wq_first", "Wq_last", "W_visited",
                      "W_load", "b_load")}
    # [h, (wf, wl, wg, wv)] stacking for the single W_all tile
    w_stack = np.ascontiguousarray(np.stack(
        [w_bf["Wq_first"], w_bf["Wq_last"], w_bf["Wq_graph"],
         w_bf["W_visited"]], axis=1))                       # [H, 4, H]
    w_lb = np.ascontiguousarray(np.stack(
        [w_bf["W_load"], w_bf["b_load"]], axis=0))[None]    # [1, 2, H]

    in_maps = []
    for c in range(NCORES):
        s = slice(c * NB, (c + 1) * NB)
        lastc = last[s]                                     # [NB, P]
        # flat gather index within the core slab: n + N*local_b
        idx = (lastc + (np.arange(NB) * N)[:, None]).astype(np.int32)
        idxt = np.ascontiguousarray(
            idx.reshape(NPAIR, 128).T)                      # [128, NPAIR]
        eqc = np.ascontiguousarray(
            eq1T[c * NPAIR:(c + 1) * NPAIR].transpose(1, 0, 2)).reshape(
            128, NPAIR * 128)
        lec = np.ascontiguousarray(
            leT[c * NPAIR:(c + 1) * NPAIR].transpose(1, 0, 2)).reshape(
            128, NPAIR * 128)
        cbig = np.ascontiguousarray(np.concatenate(
            [w_stack.reshape(128, 4 * H), eqc, lec], axis=1))
        crow = np.ascontiguousarray(np.concatenate(
            [w_lb.reshape(1, 2 * H), load[s].reshape(1, NPAIR * 128)],
            axis=1))
        in_maps.append(dict(
            dists=dists_h[s].reshape(NB * N, N),
            embt=np.ascontiguousarray(
                embt[s].transpose(1, 0, 2, 3)).reshape(128, NB * NC * H),
            maskT=np.ascontiguousarray(
                maskTh[s].transpose(1, 0, 2, 3)).reshape(128, NB * NC * 66),
            maskn=np.ascontiguousarray(
                mknpre[s].reshape(NPAIR, 128, N).transpose(1, 0, 2))
                .reshape(128, NPAIR * N),
            c_big=cbig,
            c_row=crow,
            idxt=idxt,
        ))
    return in_maps


def _run(inputs, trace=False, **kw):
    nc = _get_nc()
    in_maps = _prep_inputs(inputs)
    res = run_bass_kernel_spmd(nc, in_maps, list(range(NCORES)),
                               trace=trace, **kw)
    out = np.concatenate(
        [np.asarray(r["probs"]).astype(np.float32).reshape(NB, P, N)
         for r in res.results], axis=0)
    return out, res


def kernel(**inputs) -> np.ndarray:
    out, _ = _run(inputs)
    return out
